# revision 1
# baseline (speedup 1.0000x reference)
"""Causal multi-head attention (B=64, T=256, C=384, H=6, D=64) on 8 TRN2 cores.

Strategy: data-parallel over batch (8 batches/core). Per (batch, head) the
attention is computed transposed -- S^T = K Q^T in [s, t] layout -- so softmax
row-sums come free from an interleaved [V|ones] matmul (Y^T rows + replicated
row-sum rows in one PSUM tile) and no PE transposes are needed anywhere.
QKV/score/projection matmuls run fp32r (fp32 rounded to 11-bit mantissa;
full PE rate at free-dim >= 256); the attention-weight (P) path runs bf16.

Algebraic folds (host-side):
  - K-bias and the q.b_k term cancel in row-softmax -> only Q carries bias,
    and the 1/sqrt(D) scale is folded into W_q and b_q.
  - V-bias passes through attention (softmax rows sum to 1) ->
    b_eff = b_proj + b_v @ W_proj, added during the projection evacuation.
"""
import sys

for _p in ("/opt/trn_rl_repo", "/root/.axon_site/_ro/trn_rl_repo"):
    if _p not in sys.path:
        sys.path.insert(0, _p)

import numpy as np

N_CORES = 8
B, T, C = 64, 256, 384
H, D = 6, 64
BS = B // N_CORES  # batches per core

_compiled = None


def _round_fp32r(x: np.ndarray) -> np.ndarray:
    """Round-to-nearest-even fp32 -> fp32r (11-bit mantissa), matching HW."""
    u = np.ascontiguousarray(x, dtype=np.float32).view(np.uint32).astype(np.uint64)
    lsb = (u >> 12) & 1
    u2 = ((u + 0x7FF + lsb) & 0xFFFFF000).astype(np.uint32)
    return u2.view(np.float32)


def _build():
    import concourse.bass as bass
    import concourse.bacc as bacc
    import concourse.tile as tile
    from concourse import mybir

    F32 = mybir.dt.float32
    F32R = mybir.dt.float32r
    BF16 = mybir.dt.bfloat16
    AF = mybir.ActivationFunctionType

    nc = bacc.Bacc(None)

    xt = nc.dram_tensor("xt", [BS, C, T], F32R, kind="ExternalInput")
    wq = nc.dram_tensor("wq", [C, 3 * C], F32R, kind="ExternalInput")
    wp = nc.dram_tensor("wp", [C, C], F32R, kind="ExternalInput")
    bqs = nc.dram_tensor("bqs", [128, 3], F32, kind="ExternalInput")
    beff = nc.dram_tensor("beff", [128, C], F32, kind="ExternalInput")
    mk = nc.dram_tensor("mk", [128, 2 * T], BF16, kind="ExternalInput")
    ones_d = nc.dram_tensor("ones_d", [128, C], BF16, kind="ExternalInput")
    y = nc.dram_tensor("y", [BS, T, C], F32, kind="ExternalOutput")

    with tile.TileContext(nc) as tc:
        with (
            tc.tile_pool(name="consts", bufs=1) as consts,
            tc.tile_pool(name="vperm", bufs=1) as vperm,
            tc.tile_pool(name="xts", bufs=3) as p_xts,
            tc.tile_pool(name="qkt", bufs=15) as p_qkt,
            tc.tile_pool(name="ptmp", bufs=4) as p_ptmp,
            tc.tile_pool(name="pr", bufs=4) as p_pr,
            tc.tile_pool(name="rbt", bufs=4) as p_rbt,
            tc.tile_pool(name="yct", bufs=6) as p_yct,
            tc.tile_pool(name="ysb", bufs=3) as p_ysb,
            tc.tile_pool(name="ps_big", bufs=3, space="PSUM") as ps_big,
            tc.tile_pool(name="ps_vy", bufs=2, space="PSUM") as ps_vy,
            tc.tile_pool(name="ps_yt", bufs=3, space="PSUM") as ps_yt,
        ):
            # ---- constants ----
            # batch-0 x load + Q-weights first: they gate the first matmuls
            xts0 = p_xts.tile([128, 3 * T], F32R, tag="xts", name="xts0")
            nc.sync.dma_start(
                out=xts0, in_=xt[0].rearrange("(j p) t -> p j t", p=128))
            bqs_sb = consts.tile([128, 3], F32, tag="bqs")
            nc.sync.dma_start(out=bqs_sb, in_=bqs[:, :])
            wq_sb, wp_sb = [], []
            for i in range(3):
                t_ = consts.tile([128, 3 * C], F32R, tag=f"wq{i}")
                wq_sb.append(t_)
            for lo, hi in ((0, C), (C, 2 * C), (2 * C, 3 * C)):
                for i in range(3):
                    nc.sync.dma_start(
                        out=wq_sb[i][:, lo:hi],
                        in_=wq[i * 128:(i + 1) * 128, lo:hi],
                    )
            # later-needed consts go on the ACT HWDGE queue (parallel issue)
            mk_sb = consts.tile([128, 2 * T], BF16, tag="mk")
            nc.scalar.dma_start(out=mk_sb, in_=mk[:, :])
            vaug = [[None, None], [None, None], [None, None]]
            for par in range(3):
                for sc in range(2):
                    t_ = vperm.tile([128, 6 * 128], BF16, tag=f"vaug{par}{sc}")
                    vaug[par][sc] = t_
                    dst = bass.AP(
                        tensor=t_.tensor,
                        offset=t_[:, :].offset + 64,
                        ap=[t_[:, :].ap[0], [256, 3], [64, 2], [1, 64]],
                    )
                    nc.scalar.dma_start(out=dst, in_=ones_d[:, :])
            for i in range(3):
                t2 = consts.tile([128, C], F32R, tag=f"wp{i}")
                nc.scalar.dma_start(out=t2, in_=wp[i * 128:(i + 1) * 128, :])
                wp_sb.append(t2)
            beff_sb = consts.tile([128, C], F32, tag="beff")
            nc.scalar.dma_start(out=beff_sb, in_=beff[:, :])

            # ---- per-batch pipeline (software-pipelined emission) ----
            state = {}

            def phase_qkv(b):
                par = b % 3
                if b == 0:
                    xts = xts0
                else:
                    xts = p_xts.tile([128, 3 * T], F32R, tag="xts",
                                     name=f"xts{b}")
                    nc.sync.dma_start(
                        out=xts,
                        in_=xt[b].rearrange("(j p) t -> p j t", p=128),
                    )
                qk_ps = []
                for jj in range(3):
                    pq = ps_big.tile([128, 2 * T], F32, tag="big",
                                     name=f"pq{b}_{jj}")
                    qk_ps.append(pq)
                    for half in range(2):
                        j = 2 * jj + half
                        for i in range(3):
                            nc.tensor.matmul(
                                pq[:, half * T:(half + 1) * T],
                                wq_sb[i][:, j * 128:(j + 1) * 128],
                                xts[:, i * T:(i + 1) * T],
                                start=(i == 0),
                                stop=(i == 2),
                            )
                qt = []
                for j in range(3):
                    dst = p_qkt.tile([128, T], F32R, tag="qkt",
                                     name=f"qt{b}_{j}")
                    nc.scalar.activation(
                        out=dst,
                        in_=qk_ps[j // 2][:, (j % 2) * T:(j % 2 + 1) * T],
                        func=AF.Identity, bias=bqs_sb[:, j:j + 1], scale=1.0,
                    )
                    qt.append(dst)
                kt3 = p_qkt.tile([128, T], F32R, tag="qkt", name=f"kt3_{b}")
                nc.scalar.activation(out=kt3, in_=qk_ps[1][:, T:2 * T],
                                     func=AF.Copy)
                kt45 = p_qkt.tile([128, 2 * T], F32R, tag="qkt2",
                                  name=f"kt45_{b}")
                nc.scalar.activation(out=kt45, in_=qk_ps[2][:, :], func=AF.Copy)

                for sc in range(2):
                    pv = ps_vy.tile([128, C], F32, tag="vy",
                                    name=f"pv{b}_{sc}")
                    for i in range(3):
                        nc.tensor.matmul(
                            pv,
                            xts[:, i * T + sc * 128:i * T + (sc + 1) * 128],
                            wq_sb[i][:, 2 * C:3 * C],
                            start=(i == 0),
                            stop=(i == 2),
                        )
                    vt = vaug[par][sc]
                    dst = bass.AP(
                        tensor=vt.tensor, offset=vt[:, :].offset,
                        ap=[vt[:, :].ap[0], [256, 3], [192, 2], [1, 64]],
                    )
                    srcap = bass.AP(
                        tensor=pv.tensor, offset=pv[:, :].offset,
                        ap=[pv[:, :].ap[0], [128, 3], [64, 2], [1, 64]],
                    )
                    nc.scalar.activation(out=dst, in_=srcap, func=AF.Copy)
                state[b] = (qt, kt3, kt45)

            def phase_heads(b):
                par = b % 3
                qt, kt3, kt45 = state[b]

                def kh_ap(h):
                    rb_ = (h % 2) * 64
                    hw = h // 2
                    if hw == 0:
                        return kt3[rb_:rb_ + 64, :]
                    return kt45[rb_:rb_ + 64, (hw - 1) * T:hw * T]

                yct = [
                    p_yct.tile([128, T], F32R, tag="yct", name=f"yct{b}_{j}")
                    for j in range(3)
                ]
                for h in range(6):
                    rbase = (h % 2) * 64
                    qh = qt[h // 2][rbase:rbase + 64, :]
                    kh = kh_ap(h)

                    pst = ps_big.tile([128, 2 * T], F32, tag="big",
                                      name=f"pst{b}_{h}")
                    for sc in range(2):
                        nc.tensor.matmul(
                            pst[:, sc * T:(sc + 1) * T],
                            kh[:, sc * 128:(sc + 1) * 128],
                            qh,
                            start=True,
                            stop=True,
                        )
                    ptmp = p_ptmp.tile([128, 2 * T], BF16, tag="ptmp",
                                       name=f"ptmp{b}_{h}")
                    nc.scalar.activation(out=ptmp, in_=pst, func=AF.Exp)
                    pr = p_pr.tile([128, 2 * T], BF16, tag="pr",
                                   name=f"pr{b}_{h}")
                    nc.vector.tensor_mul(pr, ptmp, mk_sb)

                    pyt = ps_yt.tile([128, T], F32, tag="yt",
                                     name=f"pyt{b}_{h}")
                    nc.tensor.matmul(
                        pyt,
                        vaug[par][0][:, h * 128:(h + 1) * 128],
                        pr[:, 0:T],
                        start=True,
                        stop=True,
                    )
                    nc.tensor.matmul(
                        pyt[:, 128:T],
                        vaug[par][1][:, h * 128:(h + 1) * 128],
                        pr[:, T + 128:2 * T],
                        start=False,
                        stop=True,
                    )
                    ybase, sbase = (0, 64) if h % 2 == 0 else (64, 0)
                    rbt = p_rbt.tile([128, T], F32R, tag="rbt",
                                     name=f"rbt{b}_{h}")
                    with nc.allow_low_precision(reason="softmax reciprocal"):
                        nc.vector.reciprocal(
                            out=rbt[rbase:rbase + 64, :],
                            in_=pyt[sbase:sbase + 64, :],
                        )
                    nc.vector.tensor_mul(
                        yct[h // 2][rbase:rbase + 64, :],
                        pyt[ybase:ybase + 64, :],
                        rbt[rbase:rbase + 64, :],
                    )
                state[b] = yct

            def phase_proj(b):
                yct = state.pop(b)
                ysb = p_ysb.tile([128, 2 * C], F32, tag="ysb",
                                 name=f"ysb{b}")
                for tck in range(2):
                    py = ps_vy.tile([128, C], F32, tag="vy",
                                    name=f"py{b}_{tck}")
                    for j in range(3):
                        nc.tensor.matmul(
                            py,
                            yct[j][:, tck * 128:(tck + 1) * 128],
                            wp_sb[j][:, :],
                            start=(j == 0),
                            stop=(j == 2),
                        )
                    nc.vector.tensor_add(
                        ysb[:, tck * C:(tck + 1) * C], py, beff_sb)
                nc.sync.dma_start(
                    out=y[b].rearrange("(tc p) c -> p tc c", p=128),
                    in_=ysb,
                )

            phase_qkv(0)
            phase_qkv(1)
            for b in range(BS):
                if b + 2 < BS:
                    phase_qkv(b + 2)
                phase_heads(b)
                phase_proj(b)

    nc.compile()
    return nc


def _get_compiled():
    global _compiled
    if _compiled is None:
        _compiled = _build()
    return _compiled


def _make_in_maps(x, W_qkv, b_qkv, W_proj, b_proj):
    x = np.asarray(x, dtype=np.float32)
    W_qkv = np.asarray(W_qkv, dtype=np.float32)
    b_qkv = np.asarray(b_qkv, dtype=np.float32)
    W_proj = np.asarray(W_proj, dtype=np.float32)
    b_proj = np.asarray(b_proj, dtype=np.float32)

    wq_mod = W_qkv.copy()
    wq_mod[:, :C] *= 0.125                      # fold attn scale into W_q
    wq_r = _round_fp32r(wq_mod)
    wp_r = _round_fp32r(W_proj)
    bqs = np.ascontiguousarray(
        (0.125 * b_qkv[:C]).reshape(3, 128).T, dtype=np.float32
    )
    beff = np.ascontiguousarray(
        np.broadcast_to(b_proj + b_qkv[2 * C:] @ W_proj, (128, C)),
        dtype=np.float32,
    )
    ti = np.arange(T)
    m0 = (ti[None, :] >= np.arange(128)[:, None]).astype(np.float32)
    m1 = (ti[None, :] >= (128 + np.arange(128))[:, None]).astype(np.float32)
    import ml_dtypes
    mk = np.ascontiguousarray(
        np.concatenate([m0, m1], axis=1)).astype(ml_dtypes.bfloat16)
    ones_d = np.ones((128, C), dtype=ml_dtypes.bfloat16)

    in_maps = []
    for c in range(N_CORES):
        xs = x[c * BS:(c + 1) * BS]                      # [BS, T, C]
        xtr = _round_fp32r(np.ascontiguousarray(xs.transpose(0, 2, 1)))
        in_maps.append({
            "xt": xtr, "wq": wq_r, "wp": wp_r, "bqs": bqs,
            "beff": beff, "mk": mk, "ones_d": ones_d,
        })
    return in_maps


def kernel(x, W_qkv, b_qkv, W_proj, b_proj):
    nc = _get_compiled()
    from concourse.bass_utils import run_bass_kernel_spmd

    in_maps = _make_in_maps(x, W_qkv, b_qkv, W_proj, b_proj)
    res = run_bass_kernel_spmd(nc, in_maps, core_ids=list(range(N_CORES)))
    out = np.concatenate([res.results[c]["y"] for c in range(N_CORES)], axis=0)
    return out.astype(np.float32)



# revision 20
# speedup vs baseline: 1.2381x; 1.2381x over previous
"""Causal multi-head attention (B=64, T=256, C=384, H=6, D=64) on 8 TRN2 cores.

Strategy: data-parallel over batch (8 batches/core). Per (batch, head) the
attention is computed transposed -- S^T = K Q^T in [s, t] layout. Work is
spread across all four compute engines:

  PE   : QKV matmuls (bf16), scores (bf16, causally-restricted regions),
         AV with interleaved [V|ones] stationary (row sums for free), proj
         (fp32r).
  ACT  : exp, Q-bias evacuations, K1K2/V evacuations.
  DVE  : reciprocal, softmax normalization muls, K0 evacuation, proj
         bias-add evacuations.
  Pool : causal triangle masking (in-place bf16 muls).

Causal structure: for s-chunk0 (s<128) all t>=128 are kept and t<128 is a
triangle; for s-chunk1 only t>=128 exists (triangle). Scores are computed
only on the 384 needed columns per head, and masking touches just two
[128,128] triangles.

Algebraic folds (host-side):
  - K-bias and the q.b_k term cancel in row-softmax -> only Q carries bias,
    and the 1/sqrt(D) scale is folded into W_q and b_q.
  - V-bias passes through attention (softmax rows sum to 1) ->
    b_eff = b_proj + b_v @ W_proj, added during the projection evacuation.
"""
import sys

for _p in ("/opt/trn_rl_repo", "/root/.axon_site/_ro/trn_rl_repo"):
    if _p not in sys.path:
        sys.path.insert(0, _p)

import numpy as np

N_CORES = 8
B, T, C = 64, 256, 384
H, D = 6, 64
BS = B // N_CORES  # batches per core

_compiled = None


def _round_fp32r(x: np.ndarray) -> np.ndarray:
    """Round-to-nearest-even fp32 -> fp32r (11-bit mantissa), matching HW."""
    u = np.ascontiguousarray(x, dtype=np.float32).view(np.uint32).astype(np.uint64)
    lsb = (u >> 12) & 1
    u2 = ((u + 0x7FF + lsb) & 0xFFFFF000).astype(np.uint32)
    return u2.view(np.float32)


def _build():
    import concourse.bass as bass
    import concourse.bacc as bacc
    import concourse.tile as tile
    from concourse import mybir

    F32 = mybir.dt.float32
    F32R = mybir.dt.float32r
    BF16 = mybir.dt.bfloat16
    AF = mybir.ActivationFunctionType

    nc = bacc.Bacc(None)

    xt = nc.dram_tensor("xt", [BS, C, T], BF16, kind="ExternalInput")
    wq = nc.dram_tensor("wq", [C, 3 * C], BF16, kind="ExternalInput")
    wp = nc.dram_tensor("wp", [C, C], F32R, kind="ExternalInput")
    bqs = nc.dram_tensor("bqs", [128, 3], F32, kind="ExternalInput")
    beff = nc.dram_tensor("beff", [128, C], F32, kind="ExternalInput")
    mk = nc.dram_tensor("mk", [128, 128], BF16, kind="ExternalInput")
    ones_d = nc.dram_tensor("ones_d", [128, C], BF16, kind="ExternalInput")
    y = nc.dram_tensor("y", [BS, T, C], F32, kind="ExternalOutput")

    with tile.TileContext(nc) as tc:
        with (
            tc.tile_pool(name="consts", bufs=1) as consts,
            tc.tile_pool(name="vperm", bufs=1) as vperm,
            tc.tile_pool(name="xts", bufs=4) as p_xts,
            tc.tile_pool(name="qt", bufs=12) as p_qt,
            tc.tile_pool(name="kt", bufs=6) as p_kt,
            tc.tile_pool(name="pr", bufs=8) as p_pr,
            tc.tile_pool(name="rbt", bufs=6) as p_rbt,
            tc.tile_pool(name="yct", bufs=9) as p_yct,
            tc.tile_pool(name="ysb", bufs=4) as p_ysb,
            tc.tile_pool(name="ps_a", bufs=4, space="PSUM") as ps_a,
            tc.tile_pool(name="ps_vy", bufs=2, space="PSUM") as ps_vy,
            tc.tile_pool(name="ps_m", bufs=2, space="PSUM") as ps_m,
        ):
            # ---- constants ----
            # batch-0 x load + QKV weights first: they gate the first matmuls
            xts0 = p_xts.tile([128, 3 * T], BF16, tag="xts", name="xts0")
            nc.sync.dma_start(
                out=xts0, in_=xt[0].rearrange("(j p) t -> p j t", p=128))
            wq_sb = []
            for i in range(3):
                t_ = consts.tile([128, 3 * C], BF16, tag=f"wq{i}")
                wq_sb.append(t_)
            for i in range(3):
                q = nc.sync if i == 0 else nc.gpsimd
                q.dma_start(out=wq_sb[i], in_=wq[i * 128:(i + 1) * 128, :])
            # later-needed consts go on the ACT HWDGE queue (parallel issue)
            bqs_sb = consts.tile([128, 3], F32, tag="bqs")
            nc.sync.dma_start(out=bqs_sb, in_=bqs[:, :])
            # vaug[par][sc]: per 128-col head block: [V_h (64) | ones (64)]
            vaug = [[None, None], [None, None], [None, None]]
            for par in range(3):
                for sc in range(2):
                    t_ = vperm.tile([128, 6 * 128], BF16, tag=f"vaug{par}{sc}")
                    vaug[par][sc] = t_
                    dst = bass.AP(
                        tensor=t_.tensor,
                        offset=t_[:, :].offset + 64,
                        ap=[t_[:, :].ap[0], [128, 6], [1, 64]],
                    )
                    nc.sync.dma_start(out=dst, in_=ones_d[:, 0:C])
            mk_sb = consts.tile([128, 128], BF16, tag="mk")
            nc.sync.dma_start(out=mk_sb, in_=mk[:, :])
            wp_sb = []
            for i in range(3):
                t2 = consts.tile([128, C], F32R, tag=f"wp{i}")
                nc.sync.dma_start(out=t2, in_=wp[i * 128:(i + 1) * 128, :])
                wp_sb.append(t2)
            beff_sb = consts.tile([128, C], F32, tag="beff")
            nc.sync.dma_start(out=beff_sb, in_=beff[:, :])

            # ---- per-batch pipeline (piece-interleaved emission) ----
            # Engines execute their streams in emission order, so QKV of
            # batch b+1, heads of batch b and proj of batch b-1 are emitted
            # piece-by-piece per head slot to keep every engine fed.
            state = {}
            proj_state = {}

            def load_xts(b):
                if b >= BS:
                    return
                if b == 0:
                    state[0] = {"xts": xts0}
                    return
                xts = p_xts.tile([128, 3 * T], BF16, tag="xts",
                                 name=f"xts{b}")
                nc.sync.dma_start(
                    out=xts,
                    in_=xt[b].rearrange("(j p) t -> p j t", p=128),
                )
                state[b] = {"xts": xts}

            def qkv_piece(b, piece):
                """piece 0..2: QK pair; piece 3..4: V chunk."""
                if b >= BS or piece >= 5:
                    return
                st = state[b]
                xts = st["xts"]
                par = b % 3
                if piece < 3:
                    # pair p computes (q_p | k_p): unlocks heads 2p, 2p+1
                    p = piece
                    pq = ps_a.tile([128, 2 * T], F32, tag="psa",
                                   name=f"pq{b}_{p}")
                    for half in range(2):
                        for i in range(3):
                            nc.tensor.matmul(
                                pq[:, half * T:(half + 1) * T],
                                wq_sb[i][:, p * 256 + half * 128:
                                          p * 256 + (half + 1) * 128],
                                xts[:, i * T:(i + 1) * T],
                                start=(i == 0),
                                stop=(i == 2),
                            )
                    dst = p_qt.tile([128, T], BF16, tag="qt",
                                    name=f"qt{b}_{p}")
                    nc.scalar.activation(
                        out=dst, in_=pq[:, 0:T], func=AF.Identity,
                        bias=bqs_sb[:, p:p + 1], scale=1.0)
                    st[f"qt{p}"] = dst
                    kdst = p_kt.tile([128, T], BF16, tag="kt",
                                     name=f"kt{b}_{p}")
                    if p == 0:
                        nc.vector.tensor_copy(kdst, pq[:, T:2 * T])
                    else:
                        nc.scalar.activation(out=kdst, in_=pq[:, T:2 * T],
                                             func=AF.Copy)
                    st[f"kt{p}"] = kdst
                else:
                    sc = piece - 3
                    pv = ps_m.tile([128, C], F32, tag="psm",
                                   name=f"pv{b}_{sc}")
                    for i in range(3):
                        nc.tensor.matmul(
                            pv,
                            xts[:, i * T + sc * 128:i * T + (sc + 1) * 128],
                            wq_sb[i][:, 2 * C:3 * C],
                            start=(i == 0),
                            stop=(i == 2),
                        )
                    # scatter V head-dims into vaug blocks (cols 128h..+64)
                    vt = vaug[par][sc]
                    dst = bass.AP(
                        tensor=vt.tensor, offset=vt[:, :].offset,
                        ap=[vt[:, :].ap[0], [128, 6], [1, 64]],
                    )
                    srcap = bass.AP(
                        tensor=pv.tensor, offset=pv[:, :].offset,
                        ap=[pv[:, :].ap[0], [64, 6], [1, 64]],
                    )
                    nc.scalar.activation(out=dst, in_=srcap, func=AF.Copy)

            def heads_piece(b, h):
                par = b % 3
                st = state[b]
                if h == 0:
                    st["yct"] = [
                        p_yct.tile([128, T], F32R, tag="yct",
                                   name=f"yct{b}_{j}")
                        for j in range(3)
                    ]
                rb = 64 * (h % 2)
                kh = st[f"kt{h // 2}"][rb:rb + 64, :]
                qh = st[f"qt{h // 2}"][rb:rb + 64, :]

                pst = ps_a.tile([128, 2 * T], F32, tag="psa",
                                name=f"pst{b}_{h}")
                # s-chunk0 x all t (256 wide), s-chunk1 x t>=128 (128)
                nc.tensor.matmul(
                    pst[:, 0:T], kh[:, 0:128], qh, start=True, stop=True)
                nc.tensor.matmul(
                    pst[:, T:T + 128], kh[:, 128:256], qh[:, 128:T],
                    start=True, stop=True)
                pr = p_pr.tile([128, T + 128], BF16, tag="pr",
                               name=f"pr{b}_{h}")
                nc.scalar.activation(out=pr, in_=pst[:, 0:T + 128],
                                     func=AF.Exp)
                # causal triangles (in-place, Pool engine)
                nc.gpsimd.tensor_mul(pr[:, 0:128], pr[:, 0:128], mk_sb)
                nc.gpsimd.tensor_mul(pr[:, T:T + 128], pr[:, T:T + 128],
                                     mk_sb)

                col = 256 * (h % 2)
                if h % 2 == 0:
                    st["pvy"] = ps_vy.tile([128, 2 * T], F32, tag="vy",
                                           name=f"pvy{b}_{h // 2}")
                pvy = st["pvy"]
                nc.tensor.matmul(
                    pvy[:, col:col + T],
                    vaug[par][0][:, h * 128:(h + 1) * 128],
                    pr[:, 0:T],
                    start=True,
                    stop=True,
                )
                nc.tensor.matmul(
                    pvy[:, col + 128:col + T],
                    vaug[par][1][:, h * 128:(h + 1) * 128],
                    pr[:, T:T + 128],
                    start=False,
                    stop=True,
                )
                if h % 2 == 1:
                    i = h // 2
                    yct = st["yct"]
                    rbt = p_rbt.tile([64, 2 * T], F32, tag="rbt",
                                     name=f"rbt{b}_{i}")
                    with nc.allow_low_precision(reason="softmax recip"):
                        nc.vector.reciprocal(
                            out=rbt, in_=pvy[64:128, :])
                        nc.vector.tensor_mul(
                            yct[i][0:64, :], pvy[0:64, 0:T],
                            rbt[:, 0:T])
                        nc.vector.tensor_mul(
                            yct[i][64:128, :], pvy[0:64, T:2 * T],
                            rbt[:, T:2 * T])

            def proj_piece(b, piece):
                """piece 0/2: py matmuls; 1/3: bias-add evac; 4: y DMA."""
                if b < 0 or b >= BS or (piece >= 4 and piece < 10):
                    return
                if piece in (0, 10):
                    proj_state[b] = {
                        "ysb": p_ysb.tile([128, 2 * C], F32,
                                          tag="ysb", name=f"ysb{b}"),
                    }
                ps = proj_state[b]
                yct = state[b]["yct"]
                if piece in (0, 2):
                    tck = piece // 2
                    py = ps_m.tile([128, C], F32, tag="psm",
                                   name=f"py{b}_{tck}")
                    for j in range(3):
                        nc.tensor.matmul(
                            py,
                            yct[j][:, tck * 128:(tck + 1) * 128],
                            wp_sb[j][:, :],
                            start=(j == 0),
                            stop=(j == 2),
                        )
                    ps[f"py{tck}"] = py
                elif piece in (10, 11, 12):
                    # epilogue form: accumulate pair j into both py tiles as
                    # soon as its yct chunk is normalized
                    j = piece - 10
                    if j == 0:
                        ps["py0"] = ps_m.tile([128, C], F32, tag="psm",
                                              name=f"py{b}_0")
                        ps["py1"] = ps_m.tile([128, C], F32, tag="psm",
                                              name=f"py{b}_1")
                    for tck in range(2):
                        nc.tensor.matmul(
                            ps[f"py{tck}"],
                            yct[j][:, tck * 128:(tck + 1) * 128],
                            wp_sb[j][:, :],
                            start=(j == 0),
                            stop=(j == 2),
                        )
                elif piece in (1, 3):
                    tck = piece // 2
                    nc.vector.tensor_add(
                        ps["ysb"][:, tck * C:(tck + 1) * C],
                        ps[f"py{tck}"], beff_sb)
                    nc.sync.dma_start(
                        out=y[b, tck * 128:(tck + 1) * 128, :],
                        in_=ps["ysb"][:, tck * C:(tck + 1) * C],
                    )
                    if piece == 3:
                        del proj_state[b]
                        del state[b]

            # prologue: batch 0 QKV fully, then interleaved main loop
            load_xts(0)
            load_xts(1)
            for piece in range(5):
                qkv_piece(0, piece)
            for b in range(BS):
                last = b == BS - 1
                for h in range(6):
                    qkv_piece(b + 1, h)       # pieces 0-4; h==5 no-op
                    heads_piece(b, h)
                    if h == 5:
                        load_xts(b + 2)
                    proj_piece(b - 1, h)      # pieces 0-4; h==5 no-op
            for piece in range(4):
                proj_piece(BS - 1, piece)

    nc.compile()
    return nc


def _get_compiled():
    global _compiled
    if _compiled is None:
        _compiled = _build()
    return _compiled


def _make_in_maps(x, W_qkv, b_qkv, W_proj, b_proj):
    import ml_dtypes

    x = np.asarray(x, dtype=np.float32)
    W_qkv = np.asarray(W_qkv, dtype=np.float32)
    b_qkv = np.asarray(b_qkv, dtype=np.float32)
    W_proj = np.asarray(W_proj, dtype=np.float32)
    b_proj = np.asarray(b_proj, dtype=np.float32)

    wq_mod = W_qkv.copy()
    wq_mod[:, :C] *= 0.125                      # fold attn scale into W_q
    # column reorder: [q0|k0|q1|k1|q2|k2|V] so each pair loads contiguously
    cols = []
    for p in range(3):
        cols.extend(range(p * 128, (p + 1) * 128))          # q_p
        cols.extend(range(C + p * 128, C + (p + 1) * 128))  # k_p
    cols.extend(range(2 * C, 3 * C))                        # V
    wq_bf = wq_mod[:, cols].astype(ml_dtypes.bfloat16)
    wp_r = _round_fp32r(W_proj)
    bqs = np.ascontiguousarray(
        (0.125 * b_qkv[:C]).reshape(3, 128).T, dtype=np.float32
    )
    beff = np.ascontiguousarray(
        np.broadcast_to(b_proj + b_qkv[2 * C:] @ W_proj, (128, C)),
        dtype=np.float32,
    )
    idx = np.arange(128)
    mk = (idx[None, :] >= idx[:, None]).astype(ml_dtypes.bfloat16)
    mk = np.ascontiguousarray(mk)
    ones_d = np.ones((128, C), dtype=ml_dtypes.bfloat16)

    in_maps = []
    for c in range(N_CORES):
        xs = x[c * BS:(c + 1) * BS]                      # [BS, T, C]
        xtr = np.ascontiguousarray(
            xs.transpose(0, 2, 1)).astype(ml_dtypes.bfloat16)
        in_maps.append({
            "xt": xtr, "wq": wq_bf, "wp": wp_r, "bqs": bqs,
            "beff": beff, "mk": mk, "ones_d": ones_d,
        })
    return in_maps


def kernel(x, W_qkv, b_qkv, W_proj, b_proj):
    nc = _get_compiled()
    from concourse.bass_utils import run_bass_kernel_spmd

    in_maps = _make_in_maps(x, W_qkv, b_qkv, W_proj, b_proj)
    res = run_bass_kernel_spmd(nc, in_maps, core_ids=list(range(N_CORES)))
    out = np.concatenate([res.results[c]["y"] for c in range(N_CORES)], axis=0)
    return out.astype(np.float32)


# revision 27
# speedup vs baseline: 1.2866x; 1.0391x over previous
"""Causal multi-head attention (B=64, T=256, C=384, H=6, D=64) on 8 TRN2 cores.

Strategy: data-parallel over batch (8 batches/core). Per (batch, head) the
attention is computed transposed -- S^T = K Q^T in [s, t] layout. Work is
spread across all four compute engines:

  PE   : QKV matmuls (bf16), scores (bf16, causally-restricted regions),
         AV with interleaved [V|ones] stationary (row sums for free), proj
         (fp32r).
  ACT  : exp, Q-bias evacuations, K1K2/V evacuations.
  DVE  : reciprocal, softmax normalization muls, K0 evacuation, proj
         bias-add evacuations.
  Pool : causal triangle masking (in-place bf16 muls).

Causal structure: for s-chunk0 (s<128) all t>=128 are kept and t<128 is a
triangle; for s-chunk1 only t>=128 exists (triangle). Scores are computed
only on the 384 needed columns per head, and masking touches just two
[128,128] triangles.

Algebraic folds (host-side):
  - K-bias and the q.b_k term cancel in row-softmax -> only Q carries bias,
    and the 1/sqrt(D) scale is folded into W_q and b_q.
  - V-bias passes through attention (softmax rows sum to 1) ->
    b_eff = b_proj + b_v @ W_proj, added during the projection evacuation.
"""
import sys

for _p in ("/opt/trn_rl_repo", "/root/.axon_site/_ro/trn_rl_repo"):
    if _p not in sys.path:
        sys.path.insert(0, _p)

import numpy as np

N_CORES = 8
B, T, C = 64, 256, 384
H, D = 6, 64
BS = B // N_CORES  # batches per core

_compiled = None


def _round_fp32r(x: np.ndarray) -> np.ndarray:
    """Round-to-nearest-even fp32 -> fp32r (11-bit mantissa), matching HW."""
    u = np.ascontiguousarray(x, dtype=np.float32).view(np.uint32).astype(np.uint64)
    lsb = (u >> 12) & 1
    u2 = ((u + 0x7FF + lsb) & 0xFFFFF000).astype(np.uint32)
    return u2.view(np.float32)


def _build():
    import concourse.bass as bass
    import concourse.bacc as bacc
    import concourse.tile as tile
    from concourse import mybir

    F32 = mybir.dt.float32
    F32R = mybir.dt.float32r
    BF16 = mybir.dt.bfloat16
    AF = mybir.ActivationFunctionType

    nc = bacc.Bacc(None)

    xt = nc.dram_tensor("xt", [BS, C, T], BF16, kind="ExternalInput")
    wq = nc.dram_tensor("wq", [C, 3 * C], BF16, kind="ExternalInput")
    wp = nc.dram_tensor("wp", [C, C], F32R, kind="ExternalInput")
    bqs = nc.dram_tensor("bqs", [128, 3], F32, kind="ExternalInput")
    beff = nc.dram_tensor("beff", [128, C], F32, kind="ExternalInput")
    beffr = nc.dram_tensor("beffr", [1, C], F32R, kind="ExternalInput")
    onesr = nc.dram_tensor("onesr", [1, 128], F32R, kind="ExternalInput")
    mk = nc.dram_tensor("mk", [128, 128], BF16, kind="ExternalInput")
    ones_d = nc.dram_tensor("ones_d", [128, C], BF16, kind="ExternalInput")
    y = nc.dram_tensor("y", [BS, T, C], F32, kind="ExternalOutput")

    with tile.TileContext(nc) as tc:
        with (
            tc.tile_pool(name="consts", bufs=1) as consts,
            tc.tile_pool(name="vperm", bufs=1) as vperm,
            tc.tile_pool(name="xts", bufs=4) as p_xts,
            tc.tile_pool(name="qt", bufs=12) as p_qt,
            tc.tile_pool(name="kt", bufs=6) as p_kt,
            tc.tile_pool(name="pr", bufs=8) as p_pr,
            tc.tile_pool(name="rbt", bufs=6) as p_rbt,
            tc.tile_pool(name="yct", bufs=9) as p_yct,
            tc.tile_pool(name="ysb", bufs=4) as p_ysb,
            tc.tile_pool(name="ps_a", bufs=4, space="PSUM") as ps_a,
            tc.tile_pool(name="ps_vy", bufs=2, space="PSUM") as ps_vy,
            tc.tile_pool(name="ps_m", bufs=2, space="PSUM") as ps_m,
        ):
            # ---- constants ----
            # batch-0 x load + QKV weights first: they gate the first matmuls
            xts0 = p_xts.tile([128, 3 * T], BF16, tag="xts", name="xts0")
            nc.gpsimd.dma_start(
                out=xts0, in_=xt[0].rearrange("(j p) t -> p j t", p=128))
            wq_sb = []
            for i in range(3):
                t_ = consts.tile([128, 3 * C], BF16, tag=f"wq{i}")
                wq_sb.append(t_)
            for i in range(3):
                q = nc.sync if i < 2 else nc.gpsimd
                q.dma_start(out=wq_sb[i], in_=wq[i * 128:(i + 1) * 128, :])
            # later-needed consts go on the ACT HWDGE queue (parallel issue)
            bqs_sb = consts.tile([128, 3], F32, tag="bqs")
            nc.sync.dma_start(out=bqs_sb, in_=bqs[:, :])
            # vaug[par][sc]: per 128-col head block: [V_h (64) | ones (64)]
            vaug = [[None, None], [None, None], [None, None]]

            def load_vaug_ones(par, q):
                for sc in range(2):
                    t_ = vperm.tile([128, 6 * 128], BF16, tag=f"vaug{par}{sc}")
                    vaug[par][sc] = t_
                    dst = bass.AP(
                        tensor=t_.tensor,
                        offset=t_[:, :].offset + 64,
                        ap=[t_[:, :].ap[0], [128, 6], [1, 64]],
                    )
                    q.dma_start(out=dst, in_=ones_d[:, 0:C])

            load_vaug_ones(0, nc.sync)
            mk_sb = consts.tile([128, 128], BF16, tag="mk")
            nc.sync.dma_start(out=mk_sb, in_=mk[:, :])
            wp_sb = []
            beff_sb = consts.tile([128, C], F32, tag="beff")
            beffr_sb = consts.tile([1, C], F32R, tag="beffr")
            onesr_sb = consts.tile([1, 128], F32R, tag="onesr")

            def load_late_consts():
                load_vaug_ones(1, nc.sync)
                load_vaug_ones(2, nc.gpsimd)
                for i in range(3):
                    t2 = consts.tile([128, C], F32R, tag=f"wp{i}")
                    nc.sync.dma_start(out=t2, in_=wp[i * 128:(i + 1) * 128, :])
                    wp_sb.append(t2)
                nc.gpsimd.dma_start(out=beff_sb, in_=beff[:, :])
                nc.gpsimd.dma_start(out=beffr_sb, in_=beffr[:, :])
                nc.gpsimd.dma_start(out=onesr_sb, in_=onesr[:, :])

            # ---- per-batch pipeline (piece-interleaved emission) ----
            # Engines execute their streams in emission order, so QKV of
            # batch b+1, heads of batch b and proj of batch b-1 are emitted
            # piece-by-piece per head slot to keep every engine fed.
            state = {}
            proj_state = {}

            def load_xts(b):
                if b >= BS:
                    return
                if b == 0:
                    state[0] = {"xts": xts0}
                    return
                xts = p_xts.tile([128, 3 * T], BF16, tag="xts",
                                 name=f"xts{b}")
                nc.sync.dma_start(
                    out=xts,
                    in_=xt[b].rearrange("(j p) t -> p j t", p=128),
                )
                state[b] = {"xts": xts}

            def qkv_piece(b, piece):
                """piece 0..2: QK pair; piece 3..4: V chunk."""
                if b >= BS or piece >= 5:
                    return
                st = state[b]
                xts = st["xts"]
                par = b % 3
                if piece < 3:
                    # pair p computes (q_p | k_p): unlocks heads 2p, 2p+1
                    p = piece
                    pq = ps_a.tile([128, 2 * T], F32, tag="psa",
                                   name=f"pq{b}_{p}")
                    for half in range(2):
                        for i in range(3):
                            nc.tensor.matmul(
                                pq[:, half * T:(half + 1) * T],
                                wq_sb[i][:, p * 256 + half * 128:
                                          p * 256 + (half + 1) * 128],
                                xts[:, i * T:(i + 1) * T],
                                start=(i == 0),
                                stop=(i == 2),
                            )
                    dst = p_qt.tile([128, T], BF16, tag="qt",
                                    name=f"qt{b}_{p}")
                    nc.scalar.activation(
                        out=dst, in_=pq[:, 0:T], func=AF.Identity,
                        bias=bqs_sb[:, p:p + 1], scale=1.0)
                    st[f"qt{p}"] = dst
                    kdst = p_kt.tile([128, T], BF16, tag="kt",
                                     name=f"kt{b}_{p}")
                    if p == 0:
                        nc.vector.tensor_copy(kdst, pq[:, T:2 * T])
                    else:
                        nc.scalar.activation(out=kdst, in_=pq[:, T:2 * T],
                                             func=AF.Copy)
                    st[f"kt{p}"] = kdst
                else:
                    sc = piece - 3
                    pv = ps_m.tile([128, C], F32, tag="psm",
                                   name=f"pv{b}_{sc}")
                    for i in range(3):
                        nc.tensor.matmul(
                            pv,
                            xts[:, i * T + sc * 128:i * T + (sc + 1) * 128],
                            wq_sb[i][:, 2 * C:3 * C],
                            start=(i == 0),
                            stop=(i == 2),
                        )
                    # scatter V head-dims into vaug blocks (cols 128h..+64)
                    vt = vaug[par][sc]
                    dst = bass.AP(
                        tensor=vt.tensor, offset=vt[:, :].offset,
                        ap=[vt[:, :].ap[0], [128, 6], [1, 64]],
                    )
                    srcap = bass.AP(
                        tensor=pv.tensor, offset=pv[:, :].offset,
                        ap=[pv[:, :].ap[0], [64, 6], [1, 64]],
                    )
                    nc.scalar.activation(out=dst, in_=srcap, func=AF.Copy)

            def heads_piece(b, h):
                par = b % 3
                st = state[b]
                if h == 0:
                    st["yct"] = [
                        p_yct.tile([128, T], F32R, tag="yct",
                                   name=f"yct{b}_{j}")
                        for j in range(3)
                    ]
                rb = 64 * (h % 2)
                kh = st[f"kt{h // 2}"][rb:rb + 64, :]
                qh = st[f"qt{h // 2}"][rb:rb + 64, :]

                pst = ps_a.tile([128, 2 * T], F32, tag="psa",
                                name=f"pst{b}_{h}")
                # s-chunk0 x all t (256 wide), s-chunk1 x t>=128 (128)
                nc.tensor.matmul(
                    pst[:, 0:T], kh[:, 0:128], qh, start=True, stop=True)
                nc.tensor.matmul(
                    pst[:, T:T + 128], kh[:, 128:256], qh[:, 128:T],
                    start=True, stop=True)
                pr = p_pr.tile([128, T + 128], BF16, tag="pr",
                               name=f"pr{b}_{h}")
                nc.scalar.activation(out=pr, in_=pst[:, 0:T + 128],
                                     func=AF.Exp)
                # causal triangles (in-place, Pool engine)
                nc.gpsimd.tensor_mul(pr[:, 0:128], pr[:, 0:128], mk_sb)
                nc.gpsimd.tensor_mul(pr[:, T:T + 128], pr[:, T:T + 128],
                                     mk_sb)

                col = 256 * (h % 2)
                if h % 2 == 0:
                    st["pvy"] = ps_vy.tile([128, 2 * T], F32, tag="vy",
                                           name=f"pvy{b}_{h // 2}")
                pvy = st["pvy"]
                nc.tensor.matmul(
                    pvy[:, col:col + T],
                    vaug[par][0][:, h * 128:(h + 1) * 128],
                    pr[:, 0:T],
                    start=True,
                    stop=True,
                )
                nc.tensor.matmul(
                    pvy[:, col + 128:col + T],
                    vaug[par][1][:, h * 128:(h + 1) * 128],
                    pr[:, T:T + 128],
                    start=False,
                    stop=True,
                )
                if h % 2 == 1:
                    i = h // 2
                    yct = st["yct"]
                    rbt = p_rbt.tile([64, 2 * T], F32, tag="rbt",
                                     name=f"rbt{b}_{i}")
                    with nc.allow_low_precision(reason="softmax recip"):
                        nc.vector.reciprocal(
                            out=rbt, in_=pvy[64:128, :])
                        nc.vector.tensor_mul(
                            yct[i][0:64, :], pvy[0:64, 0:T],
                            rbt[:, 0:T])
                        nc.vector.tensor_mul(
                            yct[i][64:128, :], pvy[0:64, T:2 * T],
                            rbt[:, T:2 * T])

            def proj_piece(b, piece):
                """piece 0/2: py matmuls; 1/3: bias-add evac; 4: y DMA."""
                if b < 0 or b >= BS or (piece >= 4 and piece < 10):
                    return
                if piece == 0:
                    proj_state[b] = {
                        "ysb": p_ysb.tile([128, 2 * C], F32,
                                          tag="ysb", name=f"ysb{b}"),
                    }
                ps = proj_state.get(b)
                yct = state[b]["yct"]
                if piece in (0, 2):
                    tck = piece // 2
                    last = b == BS - 1
                    py = ps_m.tile([128, C], F32, tag="psm",
                                   name=f"py{b}_{tck}")
                    if last:  # bias folded in as a 1-partition matmul
                        nc.tensor.matmul(py, onesr_sb, beffr_sb,
                                         start=True, stop=False)
                    for j in range(3):
                        nc.tensor.matmul(
                            py,
                            yct[j][:, tck * 128:(tck + 1) * 128],
                            wp_sb[j][:, :],
                            start=False if last else (j == 0),
                            stop=(j == 2),
                        )
                    ps[f"py{tck}"] = py
                elif piece in (1, 3):
                    tck = piece // 2
                    if b == BS - 1:  # bias already in PSUM; parallel evacs
                        ev = nc.scalar.activation if tck == 0 else None
                        if tck == 0:
                            nc.scalar.activation(
                                out=ps["ysb"][:, 0:C],
                                in_=ps["py0"], func=AF.Copy)
                        else:
                            nc.vector.tensor_copy(
                                ps["ysb"][:, C:2 * C], ps["py1"])
                    else:
                        nc.vector.tensor_add(
                            ps["ysb"][:, tck * C:(tck + 1) * C],
                            ps[f"py{tck}"], beff_sb)
                    nc.sync.dma_start(
                        out=y[b, tck * 128:(tck + 1) * 128, :],
                        in_=ps["ysb"][:, tck * C:(tck + 1) * C],
                    )
                    if piece == 3:
                        del proj_state[b]
                        del state[b]
            # prologue: batch 0 QKV fully, then interleaved main loop
            load_xts(0)
            load_xts(1)
            for piece in range(5):
                qkv_piece(0, piece)
            load_late_consts()
            for b in range(BS):
                last = b == BS - 1
                for h in range(6):
                    qkv_piece(b + 1, h)       # pieces 0-4; h==5 no-op
                    heads_piece(b, h)
                    if h == 5:
                        load_xts(b + 2)
                    proj_piece(b - 1, h)      # pieces 0-4; h==5 no-op
            for piece in range(4):
                proj_piece(BS - 1, piece)

    nc.compile()
    return nc


def _get_compiled():
    global _compiled
    if _compiled is None:
        _compiled = _build()
    return _compiled


def _make_in_maps(x, W_qkv, b_qkv, W_proj, b_proj):
    import ml_dtypes

    x = np.asarray(x, dtype=np.float32)
    W_qkv = np.asarray(W_qkv, dtype=np.float32)
    b_qkv = np.asarray(b_qkv, dtype=np.float32)
    W_proj = np.asarray(W_proj, dtype=np.float32)
    b_proj = np.asarray(b_proj, dtype=np.float32)

    wq_mod = W_qkv.copy()
    wq_mod[:, :C] *= 0.125                      # fold attn scale into W_q
    # column reorder: [q0|k0|q1|k1|q2|k2|V] so each pair loads contiguously
    cols = []
    for p in range(3):
        cols.extend(range(p * 128, (p + 1) * 128))          # q_p
        cols.extend(range(C + p * 128, C + (p + 1) * 128))  # k_p
    cols.extend(range(2 * C, 3 * C))                        # V
    wq_bf = wq_mod[:, cols].astype(ml_dtypes.bfloat16)
    wp_r = _round_fp32r(W_proj)
    bqs = np.ascontiguousarray(
        (0.125 * b_qkv[:C]).reshape(3, 128).T, dtype=np.float32
    )
    beff_row = (b_proj + b_qkv[2 * C:] @ W_proj).astype(np.float32)
    beff = np.ascontiguousarray(np.broadcast_to(beff_row, (128, C)))
    beffr = _round_fp32r(beff_row.reshape(1, C))
    onesr = np.ones((1, 128), dtype=np.float32)
    idx = np.arange(128)
    mk = (idx[None, :] >= idx[:, None]).astype(ml_dtypes.bfloat16)
    mk = np.ascontiguousarray(mk)
    ones_d = np.ones((128, C), dtype=ml_dtypes.bfloat16)

    in_maps = []
    for c in range(N_CORES):
        xs = x[c * BS:(c + 1) * BS]                      # [BS, T, C]
        xtr = np.ascontiguousarray(
            xs.transpose(0, 2, 1)).astype(ml_dtypes.bfloat16)
        in_maps.append({
            "xt": xtr, "wq": wq_bf, "wp": wp_r, "bqs": bqs,
            "beff": beff, "mk": mk, "ones_d": ones_d,
            "beffr": beffr, "onesr": onesr,
        })
    return in_maps


def kernel(x, W_qkv, b_qkv, W_proj, b_proj):
    nc = _get_compiled()
    from concourse.bass_utils import run_bass_kernel_spmd

    in_maps = _make_in_maps(x, W_qkv, b_qkv, W_proj, b_proj)
    res = run_bass_kernel_spmd(nc, in_maps, core_ids=list(range(N_CORES)))
    out = np.concatenate([res.results[c]["y"] for c in range(N_CORES)], axis=0)
    return out.astype(np.float32)
